# revision 24
# baseline (speedup 1.0000x reference)
"""Trainium2 Bass kernel for the AdaptiveFourierNeuralOperator problem.

Math (all derived host-side, validated vs the reference in check_fold.py):
  xc  = rfft(x, ortho)        -> folded into layer-1 weights W1R/W1I = D @ w1
  irfft                       -> folded into layer-2 weights W2R/W2I = w2 @ E
  moe-1 (u1)                  -> small psc rows flowing into layer 2
  moe-2 (u2)                  -> fully folded into per-batch layer-2 composites:
                                 W2R'_b = W2R + U0_b @ W_y   (U* = u2 coefficient
                                 slabs, W_y = u2 output rows), likewise W2I'_b and
                                 the small-row matrix wsm_b. This removes the psu
                                 matmul stage entirely.
  softmax gate + null mask    -> computed host-side, folded into per-batch slabs

Device layout: [feature, seq] on-chip. x is transposed during load via the
DMA XBAR transpose unit (dma_start_transpose), so the PE does no transposes.
The on-chip position order is permuted (DRAM row 8*m + t <-> slot 128*t + m)
by writing the relu/stk outputs through strided 3D APs; this keeps layer-2
position tiles contiguous and makes the output store a contiguous
2KB-per-partition DMA. Output is stored bf16 and upcast on host.

Sharding: data-parallel over batch, 4 batches per core on 8 cores.
"""

import sys
import types

import numpy as np
import ml_dtypes

import concourse.bass as bass
from concourse import bacc
import concourse.mybir as mybir
from concourse.bass_utils import run_bass_kernel_spmd
from concourse.tile import TileContext

B, N, C, G = 32, 2048, 256, 4
F = C // 2 + 1          # 129
LORA = 4.0
N_CORES = 8
BPC = B // N_CORES      # batches per core = 4
GRP = 1024              # rows per group
NGRP = N // GRP         # groups per batch = 2
ROWS = BPC * N          # 8192 rows per core

BF16 = mybir.dt.bfloat16
FP32 = mybir.dt.float32


# ---------------------------------------------------------------- host math
def _host_precompute(inputs):
    f64 = np.float64
    w1 = inputs["w1"].astype(f64)
    b1 = inputs["b1"].astype(f64)
    w2 = inputs["w2"].astype(f64)
    b2 = inputs["b2"].astype(f64)
    emb_w = inputs["emb_w"].astype(f64)
    emb_b = inputs["emb_b"].astype(f64)
    gf = inputs["gra_feature"].astype(f64)
    A1 = inputs["A1r"].astype(f64) + 1j * inputs["A1i"].astype(f64)
    B1 = inputs["B1r"].astype(f64) + 1j * inputs["B1i"].astype(f64)
    A2 = inputs["A2r"].astype(f64) + 1j * inputs["A2i"].astype(f64)
    B2 = inputs["B2r"].astype(f64) + 1j * inputs["B2i"].astype(f64)
    tg = inputs["time_gra"].astype(f64)

    cc = np.arange(C)[:, None].astype(f64)
    ff = np.arange(F)[None, :].astype(f64)
    ang = -2.0 * np.pi * cc * ff / C
    Dr = np.cos(ang) / np.sqrt(C)
    Di = np.sin(ang) / np.sqrt(C)
    wgt = np.full(F, 2.0); wgt[0] = 1.0; wgt[-1] = 1.0
    tt = np.arange(C)[None, :].astype(f64)
    ang2 = 2.0 * np.pi * ff.T * tt / C
    Er = (wgt[:, None] * np.cos(ang2)) / np.sqrt(C)
    Ei = (-wgt[:, None] * np.sin(ang2)) / np.sqrt(C)

    W1R = Dr @ w1[0] - Di @ w1[1]            # [C, F]
    W1I = Dr @ w1[1] + Di @ w1[0]
    W2R = w2[0] @ Er + w2[1] @ Ei            # [F, C]
    W2I = w2[0] @ Ei - w2[1] @ Er
    bias_row = b2[0] @ Er + b2[1] @ Ei       # [C]

    Dc = Dr + 1j * Di
    d1 = Dc @ B1.T                           # [C, G]
    P = A1 @ B2.T                            # [G, G]

    u1r_y = A1.real @ W2R + A1.imag @ W2I    # [G, C]
    u1i_y = -A1.imag @ W2R + A1.real @ W2I
    u2r_y = A2.real @ Er + A2.imag @ Ei
    u2i_y = -A2.imag @ Er + A2.real @ Ei
    W_y = np.concatenate([u2r_y, u2i_y], axis=0)        # [8, C]

    gra = tg @ emb_w + emb_b
    logits = gra @ gf.T
    e = np.exp(logits - logits.max(axis=1, keepdims=True))
    att = (e / e.sum(axis=1, keepdims=True)).T           # [G, B]
    att = att * (tg.sum(axis=1) != 0)[None, :]           # null mask folds in

    bf = ml_dtypes.bfloat16

    # shared constants: free = (ci, m) so c-chunk ci is cols 128ci..128ci+128
    w1a = np.concatenate([W1R[0:128, 0:128], W1R[128:256, 0:128]], axis=1).astype(bf)
    w1b = np.concatenate([W1I[0:128, 0:128], W1I[128:256, 0:128]], axis=1).astype(bf)
    b1ab = np.stack([b1[0][0:128], b1[1][0:128]], axis=1).astype(np.float32)  # [128,2]
    sf = np.zeros((10, 2), dtype=np.float32)             # col0 bias, col1 floor
    sf[0, 0] = b1[0][128]
    sf[1, 0] = b1[1][128]
    sf[:, 1] = -3.0e38
    sf[0:2, 1] = 0.0

    # per-batch slabs (att * LORA folded; null batches come out zero)
    w1c = np.zeros((B, 2, 128, 10), dtype=f64)
    w2rp = np.zeros((B, 128, C), dtype=f64)
    w2ip = np.zeros((B, 128, C), dtype=f64)
    wsmp = np.zeros((B, 16, C), dtype=f64)
    for b in range(B):
        sc = att[:, b] * LORA                 # [G]
        for ci in range(2):
            sl = slice(128 * ci, 128 * (ci + 1))
            w1c[b, ci, :, 0] = W1R[sl, 128]
            w1c[b, ci, :, 1] = W1I[sl, 128]
            w1c[b, ci, :, 2:6] = sc * d1.real[sl]
            w1c[b, ci, :, 6:10] = sc * d1.imag[sl]
        U0 = np.concatenate([sc * B2.real.T[0:128], sc * B2.imag.T[0:128]], axis=1)
        U1 = np.concatenate([-sc * B2.imag.T[0:128], sc * B2.real.T[0:128]], axis=1)
        U2 = np.zeros((10, 8), dtype=f64)
        U2[0, 0:4] = sc * B2.real[:, 128];  U2[0, 4:8] = sc * B2.imag[:, 128]
        U2[1, 0:4] = -sc * B2.imag[:, 128]; U2[1, 4:8] = sc * B2.real[:, 128]
        U2[2:6, 0:4] = sc * P.real;   U2[2:6, 4:8] = sc * P.imag
        U2[6:10, 0:4] = -sc * P.imag; U2[6:10, 4:8] = sc * P.real
        w2rp[b] = W2R[0:128] + U0 @ W_y
        w2ip[b] = W2I[0:128] + U1 @ W_y
        wsm10 = np.zeros((10, C), dtype=f64)
        wsm10[0] = W2R[128]; wsm10[1] = W2I[128]
        wsm10[2:6] = u1r_y;  wsm10[6:10] = u1i_y
        wsmp[b, 0:10] = wsm10 + U2 @ W_y
        wsmp[b, 15] = bias_row

    stkinit = np.zeros((16, GRP), dtype=bf)
    stkinit[15] = 1.0
    shared = dict(w1a=w1a, w1b=w1b, b1ab=b1ab, sf=sf, stkinit=stkinit)
    per_batch = dict(w1c=w1c.astype(bf), w2rp=w2rp.astype(bf),
                     w2ip=w2ip.astype(bf), wsmp=wsmp.astype(bf))
    return shared, per_batch


def _core_layout(per_batch, i):
    """Slice per-batch slabs for core i and flatten to the 2D layouts the
    graph expects: w1c [128, 2*BPC*10] free=(ci,b,m); w2*/wsm [*, BPC*C]."""
    s = slice(BPC * i, BPC * (i + 1))
    w1c = per_batch["w1c"][s]                       # [BPC, 2, 128, 10]
    w1c2 = np.ascontiguousarray(
        w1c.transpose(2, 1, 0, 3).reshape(128, 2 * BPC * 10))
    w2rp = np.ascontiguousarray(
        per_batch["w2rp"][s].transpose(1, 0, 2).reshape(128, BPC * C))
    w2ip = np.ascontiguousarray(
        per_batch["w2ip"][s].transpose(1, 0, 2).reshape(128, BPC * C))
    wsmp = np.ascontiguousarray(
        per_batch["wsmp"][s].transpose(1, 0, 2).reshape(16, BPC * C))
    return dict(w1c=w1c2, w2rp=w2rp, w2ip=w2ip, wsmp=wsmp)


# ---------------------------------------------------------------- device graph
_NC_CACHE = {}


def _build():
    if "nc" in _NC_CACHE:
        return _NC_CACHE["nc"]
    nc = bacc.Bacc(None, target_bir_lowering=False)

    x = nc.dram_tensor("x", [ROWS, C], BF16, kind="ExternalInput")
    # output is stored channel-major [C, ROWS]; host transposes back
    out = nc.dram_tensor("out", [C, ROWS], BF16, kind="ExternalOutput")
    d_w1a = nc.dram_tensor("w1a", [128, 256], BF16, kind="ExternalInput")
    d_w1b = nc.dram_tensor("w1b", [128, 256], BF16, kind="ExternalInput")
    d_w1c = nc.dram_tensor("w1c", [128, 2 * BPC * 10], BF16, kind="ExternalInput")
    d_w2rp = nc.dram_tensor("w2rp", [128, BPC * C], BF16, kind="ExternalInput")
    d_w2ip = nc.dram_tensor("w2ip", [128, BPC * C], BF16, kind="ExternalInput")
    d_wsmp = nc.dram_tensor("wsmp", [16, BPC * C], BF16, kind="ExternalInput")
    d_b1ab = nc.dram_tensor("b1ab", [128, 2], FP32, kind="ExternalInput")
    d_sf = nc.dram_tensor("sf", [10, 2], FP32, kind="ExternalInput")
    d_stkinit = nc.dram_tensor("stkinit", [16, GRP], BF16, kind="ExternalInput")

    RELU = mybir.ActivationFunctionType.Relu
    COPY = mybir.ActivationFunctionType.Copy

    with TileContext(nc) as tc:
        with (
            tc.tile_pool(name="const", bufs=1) as cpool,
            tc.tile_pool(name="xin", bufs=5) as xpool,
            tc.tile_pool(name="work", bufs=4) as wpool,
            tc.tile_pool(name="og", bufs=2) as iopool,
            tc.tile_pool(name="psab", bufs=3, space="PSUM") as psab,
            tc.tile_pool(name="psc", bufs=2, space="PSUM") as pscp,
            tc.tile_pool(name="psy", bufs=2, space="PSUM") as psyp,
            tc.tile_pool(name="wu", bufs=1, space="PSUM") as wupool,
        ):
            # HAM warmup: dep-free dummy matmuls so the PE clock-gate opens
            # before the first real matmul. wut comes from a vector-engine
            # memset so no DMA is on the critical path.
            wut = cpool.tile([128, 128], BF16, tag="wut")
            nc.vector.memset(wut[:, :], 1.0)
            wup = wupool.tile([128, 128], FP32, tag="wu")
            for _ in range(88):
                nc.tensor.matmul(wup[:, :], wut[:, :], wut[:, :],
                                 start=True, stop=True)

            # ---- constants into SBUF. The stk inits and layer-2 slabs go
            # first on the scalar HWDGE queue (cheap configs, land before the
            # XBAR stream hogs the DMA engines); layer-1 consts on gpsimd.
            stks = []
            for si in range(2):
                st = cpool.tile([16, GRP], BF16, tag=f"stk{si}")
                nc.scalar.dma_start(out=st[:, :], in_=d_stkinit[:, :])
                stks.append(st)
            t_wsmp = cpool.tile([16, BPC * C], BF16, tag="wsmp")
            nc.scalar.dma_start(out=t_wsmp[:, :], in_=d_wsmp[:, :])
            t_w2rp = cpool.tile([128, BPC * C], BF16, tag="w2rp")
            nc.scalar.dma_start(out=t_w2rp[:, :], in_=d_w2rp[:, :])
            t_w2ip = cpool.tile([128, BPC * C], BF16, tag="w2ip")
            nc.scalar.dma_start(out=t_w2ip[:, :], in_=d_w2ip[:, :])
            t_w1a = cpool.tile([128, 256], BF16, tag="w1a")
            nc.gpsimd.dma_start(out=t_w1a[:, :], in_=d_w1a[:, :])
            t_w1b = cpool.tile([128, 256], BF16, tag="w1b")
            nc.gpsimd.dma_start(out=t_w1b[:, :], in_=d_w1b[:, :])
            t_w1c = cpool.tile([128, 2 * BPC * 10], BF16, tag="w1c")
            nc.gpsimd.dma_start(out=t_w1c[:, :], in_=d_w1c[:, :])
            t_b1ab = cpool.tile([128, 2], FP32, tag="b1ab")
            nc.gpsimd.dma_start(out=t_b1ab[:, :], in_=d_b1ab[:, :])
            t_sf = cpool.tile([10, 2], FP32, tag="sf")
            nc.gpsimd.dma_start(out=t_sf[:, :], in_=d_sf[:, :])

            def wu(n):
                for _ in range(n):
                    nc.tensor.matmul(wup[:, :], wut[:, :], wut[:, :],
                                     start=True, stop=True)

            # ---- per-group pipeline (group = 1024 rows)
            for b in range(BPC):
                for h in range(NGRP):
                    gi = b * NGRP + h
                    base = b * N + h * GRP
                    # x transposed on load: xtile[c%128, c//128, n] = x[base+n, c]
                    xtile = xpool.tile([128, 2 * GRP], BF16, tag="xt")
                    nc.sync.dma_start_transpose(
                        out=xtile[:, :].rearrange("p (di m) -> p di m", di=2),
                        in_=x[base:base + GRP, :])
                    xt0 = xtile[:, 0:GRP]
                    xt1 = xtile[:, GRP:2 * GRP]

                    s1r = wpool.tile([128, GRP], BF16, tag="s1r")
                    s1i = wpool.tile([128, GRP], BF16, tag="s1i")
                    stk = stks[gi % 2]

                    h0 = slice(0, 512)
                    h1 = slice(512, 1024)
                    # layer 1: weight-stationary, same lhsT back-to-back
                    psa0 = psab.tile([128, 512], FP32, tag="ab")
                    psa1 = psab.tile([128, 512], FP32, tag="ab")
                    nc.tensor.matmul(psa0[:, :], t_w1a[:, 0:128], xt0[:, h0],
                                     start=True, stop=False)
                    nc.tensor.matmul(psa1[:, :], t_w1a[:, 0:128], xt0[:, h1],
                                     start=True, stop=False)
                    nc.tensor.matmul(psa0[:, :], t_w1a[:, 128:256], xt1[:, h0],
                                     start=False, stop=True)
                    nc.tensor.matmul(psa1[:, :], t_w1a[:, 128:256], xt1[:, h1],
                                     start=False, stop=True)
                    nc.scalar.activation(s1r[:, h0], psa0[:, :], RELU,
                                         bias=t_b1ab[:, 0:1])
                    nc.scalar.activation(s1r[:, h1], psa1[:, :], RELU,
                                         bias=t_b1ab[:, 0:1])
                    if gi < 2:
                        wu(4)
                    psb0 = psab.tile([128, 512], FP32, tag="ab")
                    psb1 = psab.tile([128, 512], FP32, tag="ab")
                    nc.tensor.matmul(psb0[:, :], t_w1b[:, 0:128], xt0[:, h0],
                                     start=True, stop=False)
                    nc.tensor.matmul(psb1[:, :], t_w1b[:, 0:128], xt0[:, h1],
                                     start=True, stop=False)
                    nc.tensor.matmul(psb0[:, :], t_w1b[:, 128:256], xt1[:, h0],
                                     start=False, stop=True)
                    nc.tensor.matmul(psb1[:, :], t_w1b[:, 128:256], xt1[:, h1],
                                     start=False, stop=True)
                    nc.vector.tensor_scalar(
                        s1i[:, h0], psb0[:, :], t_b1ab[:, 1:2], 0.0,
                        op0=mybir.AluOpType.add, op1=mybir.AluOpType.max)
                    nc.scalar.activation(s1i[:, h1], psb1[:, :], RELU,
                                         bias=t_b1ab[:, 1:2])
                    if gi < 2:
                        wu(4)
                    psc0 = pscp.tile([10, 512], FP32, tag="c")
                    psc1 = pscp.tile([10, 512], FP32, tag="c")
                    wc0 = t_w1c[:, 10 * b:10 * (b + 1)]
                    wc1 = t_w1c[:, BPC * 10 + 10 * b:BPC * 10 + 10 * (b + 1)]
                    nc.tensor.matmul(psc0[:, :], wc0, xt0[:, h0],
                                     start=True, stop=False)
                    nc.tensor.matmul(psc1[:, :], wc0, xt0[:, h1],
                                     start=True, stop=False)
                    nc.tensor.matmul(psc0[:, :], wc1, xt1[:, h0],
                                     start=False, stop=True)
                    nc.tensor.matmul(psc1[:, :], wc1, xt1[:, h1],
                                     start=False, stop=True)
                    nc.vector.tensor_scalar(
                        stk[0:10, h0], psc0[0:10, :],
                        t_sf[:, 0:1], t_sf[:, 1:2],
                        op0=mybir.AluOpType.add, op1=mybir.AluOpType.max)
                    nc.vector.tensor_scalar(
                        stk[0:10, h1], psc1[0:10, :],
                        t_sf[:, 0:1], t_sf[:, 1:2],
                        op0=mybir.AluOpType.add, op1=mybir.AluOpType.max)

                    # layer 2: weight-stationary, output [c, pos] (transposed)
                    for cb in range(2):
                        wsl = slice(C * b + 128 * cb, C * b + 128 * (cb + 1))
                        og = iopool.tile([128, GRP], BF16, tag="og")
                        psy0 = psyp.tile([128, 512], FP32, tag="y")
                        psy1 = psyp.tile([128, 512], FP32, tag="y")
                        nc.tensor.matmul(psy0[:, :], t_w2rp[:, wsl], s1r[:, h0],
                                         start=True, stop=False)
                        nc.tensor.matmul(psy1[:, :], t_w2rp[:, wsl], s1r[:, h1],
                                         start=True, stop=False)
                        nc.tensor.matmul(psy0[:, :], t_w2ip[:, wsl], s1i[:, h0],
                                         start=False, stop=False)
                        nc.tensor.matmul(psy1[:, :], t_w2ip[:, wsl], s1i[:, h1],
                                         start=False, stop=False)
                        nc.tensor.matmul(psy0[:, :], t_wsmp[:, wsl], stk[0:16, h0],
                                         start=False, stop=True)
                        nc.tensor.matmul(psy1[:, :], t_wsmp[:, wsl], stk[0:16, h1],
                                         start=False, stop=True)
                        if gi < 2:
                            wu(4)
                        nc.vector.tensor_copy(og[:, h0], psy0[:, :])
                        nc.scalar.activation(og[:, h1], psy1[:, :], COPY)
                        nc.scalar.dma_start(
                            out=out[128 * cb:128 * (cb + 1), base:base + GRP],
                            in_=og[:, :])

    nc.compile()
    _NC_CACHE["nc"] = nc
    return nc


# ---------------------------------------------------------------- entry points
def _make_in_maps(inputs):
    shared, per_batch = _host_precompute(inputs)
    x = np.asarray(inputs["x"], dtype=np.float32).astype(ml_dtypes.bfloat16)
    in_maps = []
    for i in range(N_CORES):
        m = dict(shared)
        m["x"] = x[BPC * i:BPC * (i + 1)].reshape(ROWS, C)
        m.update(_core_layout(per_batch, i))
        in_maps.append(m)
    return in_maps


def kernel(**inputs):
    nc = _build()
    in_maps = _make_in_maps(inputs)
    res = run_bass_kernel_spmd(nc, in_maps, core_ids=list(range(N_CORES)))
    out = np.concatenate(
        [np.ascontiguousarray(r["out"].T).reshape(BPC, N, C)
         for r in res.results], axis=0)
    return out.astype(np.float32)


def run_traced(inputs):
    """For test.py: run with NTFF profiling, return (out, exec_time_ns)."""
    _install_ntff_hook()
    import concourse.bass_utils as bass_utils
    bass_utils.upload_artifacts = lambda tmpdir: f"local:{tmpdir}"
    nc = _build()
    in_maps = _make_in_maps(inputs)
    res = run_bass_kernel_spmd(nc, in_maps, core_ids=list(range(N_CORES)),
                               trace=True)
    out = np.concatenate(
        [np.ascontiguousarray(r["out"].T).reshape(BPC, N, C)
         for r in res.results], axis=0)
    return out.astype(np.float32), res.exec_time_ns


def _install_ntff_hook():
    import antenv
    if "antenv.axon_hooks" in sys.modules:
        return
    mod = types.ModuleType("antenv.axon_hooks")
    state = {"hook": None}
    mod.set_axon_ntff_profile_hook = lambda h: state.__setitem__("hook", h)
    mod.get_axon_ntff_profile_hook = lambda: state["hook"]
    sys.modules["antenv.axon_hooks"] = mod
    antenv.axon_hooks = mod
    from trn_agent_boot.trn_boot import _ntff_profile_via_ctypes
    mod.set_axon_ntff_profile_hook(
        _ntff_profile_via_ctypes("/opt/axon/libaxon_pjrt.so"))
